# revision 12
# baseline (speedup 1.0000x reference)
"""Causal multi-head attention with RoPE on Trainium2, 8 NeuronCores.

Head-parallel sharding: 16 heads / 8 cores = 2 heads per core. Each core:
q/k/v projections for its 2 heads (128 of 1024 hidden dims), flash-style
causal attention in transposed-score layout (keys on partitions, softmax
denominator from an appended ones-column in V), row-parallel slice of the
output projection. Host sums the 8 partial (D,S) outputs.

v3 over v2:
- out-projection matmuls of block i are interleaved between the projection
  matmul groups of block i+1, so the PE never idles across a block boundary
  (kills the HAM re-throttle that cost ~100us at half clock).
- engine diet: tri-mask muls moved DVE->GpSimd; RoPE cos/sin muls run in
  bf16 (2x DVE mode); q/k raw copies moved DVE->ACT; softmax reciprocal
  taken directly from the PSUM l-row (drops 2 ACT copies/block); V^T
  PSUM->SBUF evacuation as one strided ACT copy per key chunk (both heads).
- y output in bf16 (halves the DMA write), host sums partials in f32.

Self-contained: hardcodes B=1, S=4096, D=1024, H=16, hd=64.
"""

import sys

if "/opt/trn_rl_repo" not in sys.path:
    sys.path.insert(0, "/opt/trn_rl_repo")

import numpy as np

S = 4096
D = 1024
H = 16
HD = 64
NCORE = 8
P = 128
QB = 512          # query block width
NQB = S // QB     # 8
KC = 128          # key chunk
THETA = 10000.0
PEND = 3          # AV lag (chunks) behind exp

# exp split: chunk index c (global) goes to DVE when c % DVE_EXP_MOD in DVE_EXP_PHASES
DVE_EXP_MOD = 5
DVE_EXP_PHASES = (2, 4)

# custom exp constants (deg-4 minimax for e^t on [-0.4, 0.4], leading=1)
EB3 = 4.069521
EB2 = 12.099517
EB1 = 24.194042
EB0 = 24.195307
EA4 = 0.041330267
QSCALE = 0.125 / 64.0   # folded into Wq on host

_NC_CACHE = {}


def _register_custom_ops():
    from concourse.dve_spec import Spec, Src0, C0, C1, C2, lower
    from concourse.dve_uop import DveOpSpec
    from concourse import dve_ops as dvo

    def reg(name, spec):
        if name in dvo._SUB_OPCODE_FOR_NAME:
            return next(o for o in dvo.OPS if o.name == name)
        shas = {}
        for ver in ("v3",):
            uops = lower(spec, ver=ver)
            shas[ver] = DveOpSpec(name=name, opcode=0, uops=uops,
                                  rd1_en=False).sha(ver)
        op = dvo.DveOp(name, spec, subdim=False, uops_sha=shas)
        dvo.OPS.append(op)
        dvo.CUSTOM_DVE_SPECS[name] = spec
        dvo._SUB_OPCODE_FOR_NAME[name] = dvo._CUSTOM_DVE_ROW_BASE + len(dvo.OPS) - 1
        assert dvo._SUB_OPCODE_FOR_NAME[name] < 0x20
        return op

    f = np.float32
    poly = reg("EXP_POLY3M", Spec(
        body=(((Src0 + C0) * Src0 + C1) * Src0 + C2) * Src0,
        reference=lambda in0, in1, s0, s1, imm2: (
            (((in0.astype(np.float32) + f(s0)) * in0 + f(s1)) * in0 + f(imm2))
            * in0).astype(np.float32)))

    _m = (Src0 + C1) * C0
    _b = _m * _m
    for _ in range(5):
        _b = _b * _b

    def _refp(in0, in1, s0, s1, imm2):
        x = ((in0.astype(np.float32) + f(s1)) * f(s0)).astype(np.float32)
        for _ in range(6):
            x = (x * x).astype(np.float32)
        return x

    pw = reg("ADD_SCALE_POW64", Spec(body=_b, reference=_refp))
    return poly, pw


def _build_nc():
    import concourse.bacc as bacc
    import concourse.mybir as mybir
    from concourse.tile import TileContext
    from contextlib import ExitStack

    EXP_POLY3M, ADD_SCALE_POW64 = _register_custom_ops()

    F32 = mybir.dt.float32
    BF16 = mybir.dt.bfloat16
    EXP = mybir.ActivationFunctionType.Exp

    nc = bacc.Bacc("TRN2", target_bir_lowering=False)

    xT = nc.dram_tensor("xT", [D, S], BF16, kind="ExternalInput")
    wq = nc.dram_tensor("wq", [D, P], BF16, kind="ExternalInput")
    wk = nc.dram_tensor("wk", [D, P], BF16, kind="ExternalInput")
    wv = nc.dram_tensor("wv", [D, P], BF16, kind="ExternalInput")
    wo = nc.dram_tensor("wo", [P, D], BF16, kind="ExternalInput")
    cs = nc.dram_tensor("cs", [P, S], BF16, kind="ExternalInput")
    sn = nc.dram_tensor("sn", [P, S], F32, kind="ExternalInput")
    cst = nc.dram_tensor("cst", [P, 2 * P], BF16, kind="ExternalInput")
    y = nc.dram_tensor("y", [D, S], BF16, kind="ExternalOutput")

    xTr = xT.rearrange("(o p) s -> p o s", p=P)   # [128, 8, 4096]
    wqr = wq.rearrange("(o p) m -> p o m", p=P)   # [128, 8, 128]
    wkr = wk.rearrange("(o p) m -> p o m", p=P)
    wvr = wv.rearrange("(o p) m -> p o m", p=P)

    with TileContext(nc) as tc, ExitStack() as ctx:
        con = ctx.enter_context(tc.tile_pool(name="con", bufs=1))
        xp = ctx.enter_context(tc.tile_pool(name="xp", bufs=10))
        tp = ctx.enter_context(tc.tile_pool(name="tp", bufs=3))
        ptp = ctx.enter_context(tc.tile_pool(name="ptp", bufs=PEND + 2))
        plp = ctx.enter_context(tc.tile_pool(name="plp", bufs=3))
        onp_ = ctx.enter_context(tc.tile_pool(name="onp", bufs=2))
        rlp = ctx.enter_context(tc.tile_pool(name="rlp", bufs=2))
        fsp = ctx.enter_context(tc.tile_pool(name="fsp", bufs=4))
        pp = ctx.enter_context(tc.tile_pool(name="pp", bufs=2, space="PSUM"))
        scp = ctx.enter_context(tc.tile_pool(name="scp", bufs=2, space="PSUM"))
        oap = ctx.enter_context(tc.tile_pool(name="oap", bufs=1, space="PSUM"))
        obp = ctx.enter_context(tc.tile_pool(name="obp", bufs=1, space="PSUM"))

        # ---- constants / weights ----
        wq_sb = con.tile([P, 8, P], BF16)
        nc.sync.dma_start(out=wq_sb, in_=wqr)
        wk_sb = con.tile([P, 8, P], BF16)
        nc.sync.dma_start(out=wk_sb, in_=wkr)
        wv_sb = con.tile([P, 8, P], BF16)
        nc.gpsimd.dma_start(out=wv_sb, in_=wvr)
        wo_sb = con.tile([P, D], BF16)
        nc.scalar.dma_start(out=wo_sb, in_=wo[:, :])
        cs_sb = con.tile([P, S], BF16)
        sn_sb = con.tile([P, S], F32)
        nc.sync.dma_start(out=cs_sb[:, 0:QB], in_=cs[:, 0:QB])
        nc.sync.dma_start(out=sn_sb[:, 0:QB], in_=sn[:, 0:QB])
        nc.scalar.dma_start(out=cs_sb[:, QB:S], in_=cs[:, QB:S])
        nc.scalar.dma_start(out=sn_sb[:, QB:S], in_=sn[:, QB:S])
        cst_sb = con.tile([P, 2, P], BF16)
        nc.sync.dma_start(out=cst_sb, in_=cst.rearrange("p (t m) -> p t m", t=2))
        pswap_sb = cst_sb[:, 0, :]
        tri_sb = cst_sb[:, 1, :]

        # v in [key, hd] layout, both heads + ones column at index HD
        # layout [keys, chunk, head, HD+1]; AV lhsT = vna[:, ci, h, 0:HD+1]
        vna = con.tile([P, S // KC, 2, HD + 1], BF16, tag="vna", name="vna")
        nc.vector.memset(vna[:, :, :, HD : HD + 1], 1.0)

        qTr = con.tile([P, S], BF16, tag="qTr")
        kTr = con.tile([P, S], BF16, tag="kTr")

        # HAM warmup: dependency-free matmuls on scratch tiles keep the PE
        # busy (and its clock at 8/8) while the first DMAs land.
        wsc_w = con.tile([P, P], BF16, tag="wscw")
        wsc_x = con.tile([P, QB], BF16, tag="wscx")
        nc.vector.memset(wsc_w, 1.0)
        nc.vector.memset(wsc_x, 1.0)
        wups = oap.tile([P, QB], F32, tag="oA", name="warm")
        for _ in range(40):
            nc.tensor.matmul(wups, wsc_w, wsc_x, start=True, stop=True)

        def proj_fillers(st):
            """Projection work for block st as a list of PE-filler closures,
            to be interleaved between attention chunks of block st-1."""
            sl = slice(st * QB, (st + 1) * QB)
            xts = []
            for dk in range(8):
                xt = xp.tile([P, QB], BF16, tag="x")
                nc.sync.dma_start(out=xt, in_=xTr[:, dk, sl])
                xts.append(xt)

            state = {}

            def mk_acc(key, wsb, half):
                def job():
                    if half == 0:
                        state[key] = pp.tile([P, QB], F32, tag="ps",
                                             name=f"acc{key}")
                    acc = state[key]
                    for dk in range(4 * half, 4 * half + 4):
                        nc.tensor.matmul(acc, wsb[:, dk, :], xts[dk],
                                         start=(dk == 0), stop=(dk == 7))
                    if half == 1:
                        raw = tp.tile([P, QB], BF16, tag="raw",
                                      name=f"raw{key}")
                        nc.scalar.copy(out=raw, in_=acc)
                        state[f"raw{key}"] = raw
                        t1 = tp.tile([P, QB], BF16, tag=f"t1{key}",
                                     name=f"t1{key}")
                        nc.vector.tensor_mul(out=t1, in0=raw, in1=cs_sb[:, sl])
                        state[f"t1{key}"] = t1
                return job

            def mk_rope(key, dstT):
                def job():
                    sw = pp.tile([P, QB], F32, tag="ps", name=f"sw{key}")
                    nc.tensor.matmul(sw, pswap_sb, state[f"raw{key}"],
                                     start=True, stop=True)
                    t2 = tp.tile([P, QB], BF16, tag="t2")
                    nc.vector.tensor_mul(out=t2, in0=sw, in1=sn_sb[:, sl])
                    nc.vector.tensor_add(out=dstT[:, sl], in0=state[f"t1{key}"],
                                         in1=t2)
                return job

            def mk_vac(sub):
                def job():
                    vac = pp.tile([P, QB], F32, tag="ps", name="vac")[:, 0:KC]
                    for dk in range(8):
                        nc.tensor.matmul(vac,
                                         xts[dk][:, sub * KC : (sub + 1) * KC],
                                         wv_sb[:, dk, :],
                                         start=(dk == 0), stop=(dk == 7))
                    ci = st * 4 + sub
                    nc.scalar.copy(out=vna[:, ci, :, 0:HD], in_=vac[:, 0:P])
                return job

            return ([mk_acc("q", wq_sb, 0), mk_acc("q", wq_sb, 1),
                     mk_acc("k", wk_sb, 0), mk_acc("k", wk_sb, 1),
                     mk_rope("q", qTr), mk_rope("k", kTr)]
                    + [mk_vac(s) for s in range(4)])

        chunk_counter = [0]

        def emit_attn(qsb, fillers, tail_fillers):
            """Scores + exp + AV for query block qsb. `fillers` (next block's
            q/k proj + RoPE) are interleaved between chunks so the PE stream
            stays dense; `tail_fillers` (next block's V^T) are emitted after
            the epilogue to cover its latency before the out-projection."""
            nch = 4 * (qsb + 1)
            fillers = list(fillers)
            oA = oap.tile([P, QB], F32, tag="oA")
            oB = obp.tile([P, QB], F32, tag="oB")
            pend = []

            def flush_av(last):
                ppt, poff, pc = pend.pop(0)
                for h, o in ((0, oA), (1, oB)):
                    nc.tensor.matmul(
                        o[0 : HD + 1, poff:QB],
                        vna[:, pc, h, :],
                        ppt[:, h, poff:QB],
                        start=(pc == 0), stop=last and (pc == nch - 1),
                    )

            nfill = len(fillers)
            for c in range(nch):
                # pop fillers so they are spread over the block's chunks
                want = min(nfill, ((c + 1) * nfill + nch - 1) // nch)
                while nfill - len(fillers) < want:
                    fillers.pop(0)()
                is_diag = (c // 4) == qsb
                off = (c % 4) * KC if is_diag else 0
                sp = scp.tile([P, 2, QB], F32, tag="sc")
                for h in (0, 1):
                    nc.tensor.matmul(
                        sp[:, h, off:QB],
                        kTr[h * HD : (h + 1) * HD, c * KC : (c + 1) * KC],
                        qTr[h * HD : (h + 1) * HD, qsb * QB + off : (qsb + 1) * QB],
                        start=True, stop=True,
                        tile_position=(h * HD, 0),
                    )
                pt = ptp.tile([P, 2, QB], BF16, tag="pt")
                gc = chunk_counter[0]
                chunk_counter[0] += 1
                if (not is_diag) and gc % DVE_EXP_MOD in DVE_EXP_PHASES:
                    pl = plp.tile([P, 2, QB], F32, tag="pl")
                    nc.vector._custom_dve(
                        EXP_POLY3M, out=pl[:, :, off:QB], in0=sp[:, :, off:QB],
                        s0=EB3, s1=EB2, imm2=EB1)
                    nc.vector._custom_dve(
                        ADD_SCALE_POW64, out=pt[:, :, off:QB], in0=pl[:, :, off:QB],
                        s0=EA4, s1=EB0)
                else:
                    nc.scalar.activation(
                        out=pt[:, :, off:QB], in_=sp[:, :, off:QB], func=EXP,
                        scale=64.0)
                if is_diag:
                    for h in (0, 1):
                        nc.gpsimd.tensor_mul(
                            out=pt[:, h, off : off + KC],
                            in0=pt[:, h, off : off + KC],
                            in1=tri_sb,
                        )
                if len(pend) == PEND:
                    flush_av(False)
                pend.append((pt, off, c))
            while fillers:
                fillers.pop(0)()
            while pend:
                flush_av(True)

            # epilogue: l-rows to SBUF on ACT (cross-partition copy), DVE
            # reciprocal on the [1,QB] rows, gpsimd broadcast of the recip.
            rlA = rlp.tile([1, QB], F32, tag="rlA")
            nc.scalar.copy(out=rlA, in_=oA[HD : HD + 1, :])
            rlB = rlp.tile([1, QB], F32, tag="rlB")
            nc.scalar.copy(out=rlB, in_=oB[HD : HD + 1, :])
            rA = rlp.tile([1, QB], F32, tag="rA")
            nc.vector.reciprocal_approx_fast(out=rA, in_=rlA)
            rB = rlp.tile([1, QB], F32, tag="rB")
            nc.vector.reciprocal_approx_fast(out=rB, in_=rlB)
            rlbA = onp_.tile([HD, QB], F32, tag="rlb")
            nc.gpsimd.partition_broadcast(rlbA[0:HD, :], rA[0:1, :])
            rlbB = onp_.tile([HD, QB], F32, tag="rlb")
            nc.gpsimd.partition_broadcast(rlbB[0:HD, :], rB[0:1, :])

            onT = onp_.tile([P, QB], BF16, tag="onT")
            nc.vector.tensor_mul(out=onT[0:HD, :], in0=oA[0:HD, :], in1=rlbA[0:HD, :])
            oBn = onp_.tile([HD, QB], BF16, tag="oBn")
            nc.vector.tensor_mul(out=oBn[0:HD, :], in0=oB[0:HD, :], in1=rlbB[0:HD, :])
            nc.gpsimd.dma_start(out=onT[HD : 2 * HD, :], in_=oBn[0:HD, :])

            # next block's V^T fills the PE while the epilogue chain runs
            for job in tail_fillers:
                job()

            # out-projection slice
            for jc in range(8):
                fp = (oap if jc % 2 == 0 else obp).tile(
                    [P, QB], F32, tag="oA" if jc % 2 == 0 else "oB", name="fp")
                nc.tensor.matmul(fp, wo_sb[:, jc * P : (jc + 1) * P], onT,
                                 start=True, stop=True)
                fs = fsp.tile([P, QB], BF16, tag="fs")
                if jc % 2 == 0:
                    nc.scalar.copy(out=fs, in_=fp)
                else:
                    nc.vector.tensor_copy(out=fs, in_=fp)
                nc.sync.dma_start(
                    out=y[jc * P : (jc + 1) * P, qsb * QB : (qsb + 1) * QB],
                    in_=fs)

        # pipeline: proj(0) upfront; then attn(i) with proj(i+1)'s q/k work
        # spread between chunks and its V^T work after the epilogue.
        for job in proj_fillers(0):
            job()
        for it_ in range(NQB):
            if it_ + 1 < NQB:
                pf = proj_fillers(it_ + 1)
                mid, tail = pf[:6], pf[6:]
            else:
                mid, tail = [], []
            emit_attn(it_, mid, tail)

    nc.compile()
    return nc


def _host_prep(x, token_positions, Wq, Wk, Wv, Wo):
    import ml_dtypes

    bf16 = ml_dtypes.bfloat16
    x = np.asarray(x, dtype=np.float32)
    pos = np.asarray(token_positions).astype(np.float32)
    Wq = np.asarray(Wq, dtype=np.float32) * np.float32(QSCALE)
    Wk = np.asarray(Wk, dtype=np.float32)
    Wv = np.asarray(Wv, dtype=np.float32)
    Wo = np.asarray(Wo, dtype=np.float32)

    xT = np.ascontiguousarray(x.reshape(S, D).T)  # [D, S]

    freqs = (1.0 / THETA ** (np.arange(0, HD, 2, dtype=np.float32) / HD)).astype(
        np.float32)
    ang = pos[:, None] * freqs[None, :]          # [S, 32]
    cosv = np.cos(ang).astype(np.float32).T      # [32, S]
    sinv = np.sin(ang).astype(np.float32).T
    C64 = np.repeat(cosv, 2, axis=0)             # [64, S]
    S64 = np.empty((HD, S), dtype=np.float32)
    S64[0::2] = -sinv
    S64[1::2] = sinv
    C = np.tile(C64, (2, 1))                     # [128, S]
    Sg = np.tile(S64, (2, 1))

    pswap = np.zeros((P, P), dtype=np.float32)
    idx = np.arange(P)
    pswap[idx ^ 1, idx] = 1.0
    tri = np.triu(np.ones((P, P), dtype=np.float32))
    cst = np.concatenate([pswap, tri], axis=1)   # [128, 256]

    def b(a):
        return np.ascontiguousarray(a).astype(bf16)

    in_maps = []
    for c in range(NCORE):
        r = slice(c * P, (c + 1) * P)
        in_maps.append({
            "xT": b(xT),
            "wq": b(Wq[r, :].T),
            "wk": b(Wk[r, :].T),
            "wv": b(Wv[r, :].T),
            "wo": b(Wo[:, r].T),
            "cs": b(C),
            "sn": Sg,
            "cst": b(cst),
        })
    return in_maps


LAST_EXEC_NS = None
LAST_TRACE = None


def kernel(x, token_positions, Wq, Wk, Wv, Wo):
    global LAST_EXEC_NS, LAST_TRACE
    from concourse.bass_utils import run_bass_kernel_spmd

    if "nc" not in _NC_CACHE:
        _NC_CACHE["nc"] = _build_nc()
    nc = _NC_CACHE["nc"]

    in_maps = _host_prep(x, token_positions, Wq, Wk, Wv, Wo)
    res = run_bass_kernel_spmd(nc, in_maps, core_ids=list(range(NCORE)))
    LAST_EXEC_NS = res.exec_time_ns
    LAST_TRACE = (
        res.instructions_and_trace[1]
        if res.instructions_and_trace is not None
        else None
    )

    acc = np.zeros((D, S), dtype=np.float32)
    for r in res.results:
        acc += r["y"].astype(np.float32)
    out = acc.T.astype(np.float32).reshape(1, S, D)
    return out


# revision 21
# speedup vs baseline: 1.1909x; 1.1909x over previous
"""Causal multi-head attention with RoPE on Trainium2, 8 NeuronCores.

Head-parallel sharding: 16 heads / 8 cores = 2 heads per core. Each core:
q/k/v projections for its 2 heads (128 of 1024 hidden dims), flash-style
causal attention in transposed-score layout (keys on partitions, softmax
denominator from an appended ones-column in V), row-parallel slice of the
output projection. Host sums the 8 partial (D,S) outputs.

v3 over v2:
- out-projection matmuls of block i are interleaved between the projection
  matmul groups of block i+1, so the PE never idles across a block boundary
  (kills the HAM re-throttle that cost ~100us at half clock).
- engine diet: tri-mask muls moved DVE->GpSimd; RoPE cos/sin muls run in
  bf16 (2x DVE mode); q/k raw copies moved DVE->ACT; softmax reciprocal
  taken directly from the PSUM l-row (drops 2 ACT copies/block); V^T
  PSUM->SBUF evacuation as one strided ACT copy per key chunk (both heads).
- y output in bf16 (halves the DMA write), host sums partials in f32.

Self-contained: hardcodes B=1, S=4096, D=1024, H=16, hd=64.
"""

import sys

if "/opt/trn_rl_repo" not in sys.path:
    sys.path.insert(0, "/opt/trn_rl_repo")

import numpy as np

S = 4096
D = 1024
H = 16
HD = 64
NCORE = 8
P = 128
QB = 512          # query block width
NQB = S // QB     # 8
KC = 128          # key chunk
THETA = 10000.0
PEND = 3          # AV lag (chunks) behind exp

# exp split: chunk index c (global) goes to DVE when c % DVE_EXP_MOD in DVE_EXP_PHASES
DVE_EXP_MOD = 4
DVE_EXP_PHASES = (3,)

# custom exp constants (deg-4 minimax for e^t on [-0.4, 0.4], leading=1)
EB3 = 4.069521
EB2 = 12.099517
EB1 = 24.194042
EB0 = 24.195307
EA4 = 0.041330267
QSCALE = 0.125 / 64.0   # folded into Wq on host
SWAP_MASK = [i ^ 1 for i in range(32)]

_NC_CACHE = {}


def _register_custom_ops():
    from concourse.dve_spec import Spec, Src0, C0, C1, C2, lower
    from concourse.dve_uop import DveOpSpec
    from concourse import dve_ops as dvo

    def reg(name, spec):
        if name in dvo._SUB_OPCODE_FOR_NAME:
            return next(o for o in dvo.OPS if o.name == name)
        shas = {}
        for ver in ("v3",):
            uops = lower(spec, ver=ver)
            shas[ver] = DveOpSpec(name=name, opcode=0, uops=uops,
                                  rd1_en=False).sha(ver)
        op = dvo.DveOp(name, spec, subdim=False, uops_sha=shas)
        dvo.OPS.append(op)
        dvo.CUSTOM_DVE_SPECS[name] = spec
        dvo._SUB_OPCODE_FOR_NAME[name] = dvo._CUSTOM_DVE_ROW_BASE + len(dvo.OPS) - 1
        assert dvo._SUB_OPCODE_FOR_NAME[name] < 0x20
        return op

    f = np.float32
    poly = reg("EXP_POLY3M", Spec(
        body=(((Src0 + C0) * Src0 + C1) * Src0 + C2) * Src0,
        reference=lambda in0, in1, s0, s1, imm2: (
            (((in0.astype(np.float32) + f(s0)) * in0 + f(s1)) * in0 + f(imm2))
            * in0).astype(np.float32)))

    _m = (Src0 + C1) * C0
    _b = _m * _m
    for _ in range(5):
        _b = _b * _b

    def _refp(in0, in1, s0, s1, imm2):
        x = ((in0.astype(np.float32) + f(s1)) * f(s0)).astype(np.float32)
        for _ in range(6):
            x = (x * x).astype(np.float32)
        return x

    pw = reg("ADD_SCALE_POW64", Spec(body=_b, reference=_refp))
    return poly, pw


def _build_nc():
    import concourse.bacc as bacc
    import concourse.mybir as mybir
    from concourse.tile import TileContext
    from contextlib import ExitStack

    EXP_POLY3M, ADD_SCALE_POW64 = _register_custom_ops()

    F32 = mybir.dt.float32
    BF16 = mybir.dt.bfloat16
    EXP = mybir.ActivationFunctionType.Exp

    nc = bacc.Bacc("TRN2", target_bir_lowering=False)

    xT = nc.dram_tensor("xT", [D, S], BF16, kind="ExternalInput")
    wq = nc.dram_tensor("wq", [D, P], BF16, kind="ExternalInput")
    wk = nc.dram_tensor("wk", [D, P], BF16, kind="ExternalInput")
    wv = nc.dram_tensor("wv", [D, P], BF16, kind="ExternalInput")
    wo = nc.dram_tensor("wo", [P, D], BF16, kind="ExternalInput")
    cs = nc.dram_tensor("cs", [P, S], BF16, kind="ExternalInput")
    sn = nc.dram_tensor("sn", [P, S], BF16, kind="ExternalInput")
    cst = nc.dram_tensor("cst", [P, 2 * P], BF16, kind="ExternalInput")
    y = nc.dram_tensor("y", [D, S], BF16, kind="ExternalOutput")

    xTr = xT.rearrange("(o p) s -> p o s", p=P)   # [128, 8, 4096]
    wqr = wq.rearrange("(o p) m -> p o m", p=P)   # [128, 8, 128]
    wkr = wk.rearrange("(o p) m -> p o m", p=P)
    wvr = wv.rearrange("(o p) m -> p o m", p=P)

    with TileContext(nc) as tc, ExitStack() as ctx:
        con = ctx.enter_context(tc.tile_pool(name="con", bufs=1))
        xp = ctx.enter_context(tc.tile_pool(name="xp", bufs=10))
        tp = ctx.enter_context(tc.tile_pool(name="tp", bufs=3))
        ptp = ctx.enter_context(tc.tile_pool(name="ptp", bufs=PEND + 2))
        plp = ctx.enter_context(tc.tile_pool(name="plp", bufs=3))
        onp_ = ctx.enter_context(tc.tile_pool(name="onp", bufs=2))
        rlp = ctx.enter_context(tc.tile_pool(name="rlp", bufs=2))
        fsp = ctx.enter_context(tc.tile_pool(name="fsp", bufs=4))
        pp = ctx.enter_context(tc.tile_pool(name="pp", bufs=2, space="PSUM"))
        scp = ctx.enter_context(tc.tile_pool(name="scp", bufs=2, space="PSUM"))
        oap = ctx.enter_context(tc.tile_pool(name="oap", bufs=1, space="PSUM"))
        obp = ctx.enter_context(tc.tile_pool(name="obp", bufs=1, space="PSUM"))

        # ---- constants / weights ----
        wq_sb = con.tile([P, 8, P], BF16)
        nc.sync.dma_start(out=wq_sb, in_=wqr)
        wk_sb = con.tile([P, 8, P], BF16)
        nc.sync.dma_start(out=wk_sb, in_=wkr)
        wv_sb = con.tile([P, 8, P], BF16)
        nc.gpsimd.dma_start(out=wv_sb, in_=wvr)
        wo_sb = con.tile([P, D], BF16)
        nc.scalar.dma_start(out=wo_sb, in_=wo[:, :])
        cs_sb = con.tile([P, S], BF16)
        sn_sb = con.tile([P, S], BF16)
        nc.sync.dma_start(out=cs_sb[:, 0:QB], in_=cs[:, 0:QB])
        nc.sync.dma_start(out=sn_sb[:, 0:QB], in_=sn[:, 0:QB])
        nc.scalar.dma_start(out=cs_sb[:, QB:S], in_=cs[:, QB:S])
        nc.scalar.dma_start(out=sn_sb[:, QB:S], in_=sn[:, QB:S])
        cst_sb = con.tile([P, 2, P], BF16)
        nc.sync.dma_start(out=cst_sb, in_=cst.rearrange("p (t m) -> p t m", t=2))
        pswap_sb = cst_sb[:, 0, :]
        tri_sb = cst_sb[:, 1, :]

        # v in [key, hd] layout, both heads + ones column at index HD
        # layout [keys, chunk, head, HD+1]; AV lhsT = vna[:, ci, h, 0:HD+1]
        vna = con.tile([P, S // KC, 2, HD + 1], BF16, tag="vna", name="vna")
        nc.vector.memset(vna[:, :, :, HD : HD + 1], 1.0)

        qTr = con.tile([P, S], BF16, tag="qTr")
        kTr = con.tile([P, S], BF16, tag="kTr")

        # HAM warmup: dependency-free matmuls on scratch tiles keep the PE
        # busy (and its clock at 8/8) while the first DMAs land.
        wsc_w = con.tile([P, P], BF16, tag="wscw")
        wsc_x = con.tile([P, QB], BF16, tag="wscx")
        nc.vector.memset(wsc_w, 1.0)
        nc.vector.memset(wsc_x, 1.0)
        wups = oap.tile([P, QB], F32, tag="oA", name="warm")
        for _ in range(40):
            nc.tensor.matmul(wups, wsc_w, wsc_x, start=True, stop=True)

        def proj_fillers(st):
            """Projection work for block st as a list of PE-filler closures,
            to be interleaved between attention chunks of block st-1."""
            sl = slice(st * QB, (st + 1) * QB)
            xts = []
            for dk in range(8):
                xt = xp.tile([P, QB], BF16, tag="x")
                nc.sync.dma_start(out=xt, in_=xTr[:, dk, sl])
                xts.append(xt)

            state = {}

            def mk_acc(key, wsb, half):
                def job():
                    if half == 0:
                        state[key] = pp.tile([P, QB], F32, tag="ps",
                                             name=f"acc{key}")
                    acc = state[key]
                    for dk in range(4 * half, 4 * half + 4):
                        nc.tensor.matmul(acc, wsb[:, dk, :], xts[dk],
                                         start=(dk == 0), stop=(dk == 7))
                    if half == 1:
                        raw = tp.tile([P, QB], BF16, tag="raw",
                                      name=f"raw{key}")
                        nc.scalar.copy(out=raw, in_=acc)
                        state[f"raw{key}"] = raw
                return job

            def mk_rope(key, dstT):
                # pair-swap via DVE quadrant shuffle (i^1 is within-quadrant),
                # all-bf16 muls/adds run in the DVE 2x mode; no PSUM tiles.
                def job():
                    raw = state[f"raw{key}"]
                    t1 = tp.tile([P, QB], BF16, tag=f"t1{key}",
                                 name=f"t1{key}")
                    nc.vector.tensor_mul(out=t1, in0=raw, in1=cs_sb[:, sl])
                    shuf = tp.tile([P, QB], BF16, tag="shuf")
                    nc.vector.stream_shuffle(out=shuf, in_=raw, mask=SWAP_MASK)
                    t2 = tp.tile([P, QB], BF16, tag="t2")
                    nc.vector.tensor_mul(out=t2, in0=shuf, in1=sn_sb[:, sl])
                    nc.vector.tensor_add(out=dstT[:, sl], in0=t1, in1=t2)
                return job

            def mk_vac(sub):
                def job():
                    vac = pp.tile([P, QB], F32, tag="ps", name="vac")[:, 0:KC]
                    for dk in range(8):
                        nc.tensor.matmul(vac,
                                         xts[dk][:, sub * KC : (sub + 1) * KC],
                                         wv_sb[:, dk, :],
                                         start=(dk == 0), stop=(dk == 7))
                    ci = st * 4 + sub
                    nc.scalar.copy(out=vna[:, ci, :, 0:HD], in_=vac[:, 0:P])
                return job

            return ([mk_acc("q", wq_sb, 0), mk_acc("q", wq_sb, 1),
                     mk_rope("q", qTr),
                     mk_acc("k", wk_sb, 0), mk_acc("k", wk_sb, 1),
                     mk_rope("k", kTr)]
                    + [mk_vac(s) for s in range(4)])

        chunk_counter = [0]

        def emit_attn(qsb, fillers, tail_fillers):
            """Scores + exp + AV for query block qsb. `fillers` (next block's
            q/k proj + RoPE) are interleaved between chunks so the PE stream
            stays dense; `tail_fillers` (next block's V^T) are emitted after
            the epilogue to cover its latency before the out-projection."""
            nch = 4 * (qsb + 1)
            fillers = list(fillers)
            oA = oap.tile([P, QB], F32, tag="oA")
            oB = obp.tile([P, QB], F32, tag="oB")
            pend = []

            def flush_av(last):
                ppt, poff, pc = pend.pop(0)
                for h, o in ((0, oA), (1, oB)):
                    nc.tensor.matmul(
                        o[0 : HD + 1, poff:QB],
                        vna[:, pc, h, :],
                        ppt[:, h, poff:QB],
                        start=(pc == 0), stop=last and (pc == nch - 1),
                    )

            for c in range(nch):
                # one filler every 2 chunks: early enough that qTr/kTr for
                # the next block are ready well before its first scores, late
                # enough that the RoPE DVE ops don't head-of-line-block the
                # DVE queue behind un-executed producers.
                if c % 2 == 0 and fillers:
                    fillers.pop(0)()
                is_diag = (c // 4) == qsb
                off = (c % 4) * KC if is_diag else 0
                sp = scp.tile([P, 2, QB], F32, tag="sc")
                for h in (0, 1):
                    nc.tensor.matmul(
                        sp[:, h, off:QB],
                        kTr[h * HD : (h + 1) * HD, c * KC : (c + 1) * KC],
                        qTr[h * HD : (h + 1) * HD, qsb * QB + off : (qsb + 1) * QB],
                        start=True, stop=True,
                        tile_position=(h * HD, 0),
                    )
                pt = ptp.tile([P, 2, QB], BF16, tag="pt")
                gc = chunk_counter[0]
                chunk_counter[0] += 1
                if (not is_diag) and gc % DVE_EXP_MOD in DVE_EXP_PHASES:
                    pl = plp.tile([P, 2, QB], F32, tag="pl")
                    nc.vector._custom_dve(
                        EXP_POLY3M, out=pl[:, :, off:QB], in0=sp[:, :, off:QB],
                        s0=EB3, s1=EB2, imm2=EB1)
                    nc.vector._custom_dve(
                        ADD_SCALE_POW64, out=pt[:, :, off:QB], in0=pl[:, :, off:QB],
                        s0=EA4, s1=EB0)
                else:
                    nc.scalar.activation(
                        out=pt[:, :, off:QB], in_=sp[:, :, off:QB], func=EXP,
                        scale=64.0)
                if is_diag:
                    for h in (0, 1):
                        nc.gpsimd.tensor_mul(
                            out=pt[:, h, off : off + KC],
                            in0=pt[:, h, off : off + KC],
                            in1=tri_sb,
                        )
                if len(pend) == PEND:
                    flush_av(False)
                pend.append((pt, off, c))
            while fillers:
                fillers.pop(0)()
            while pend:
                flush_av(True)

            # epilogue: l-rows to SBUF on ACT (cross-partition copy), DVE
            # reciprocal on the [1,QB] rows, gpsimd broadcast of the recip.
            rlA = rlp.tile([1, QB], F32, tag="rlA")
            nc.scalar.copy(out=rlA, in_=oA[HD : HD + 1, :])
            rlB = rlp.tile([1, QB], F32, tag="rlB")
            nc.scalar.copy(out=rlB, in_=oB[HD : HD + 1, :])
            rA = rlp.tile([1, QB], F32, tag="rA")
            nc.vector.reciprocal_approx_fast(out=rA, in_=rlA)
            rB = rlp.tile([1, QB], F32, tag="rB")
            nc.vector.reciprocal_approx_fast(out=rB, in_=rlB)
            rlbA = onp_.tile([HD, QB], F32, tag="rlb")
            nc.gpsimd.partition_broadcast(rlbA[0:HD, :], rA[0:1, :])
            rlbB = onp_.tile([HD, QB], F32, tag="rlb")
            nc.gpsimd.partition_broadcast(rlbB[0:HD, :], rB[0:1, :])

            onT = onp_.tile([P, QB], BF16, tag="onT")
            nc.vector.tensor_mul(out=onT[0:HD, :], in0=oA[0:HD, :], in1=rlbA[0:HD, :])
            oBn = onp_.tile([HD, QB], BF16, tag="oBn")
            nc.vector.tensor_mul(out=oBn[0:HD, :], in0=oB[0:HD, :], in1=rlbB[0:HD, :])
            nc.gpsimd.dma_start(out=onT[HD : 2 * HD, :], in_=oBn[0:HD, :])

            # next block's V^T fills the PE while the epilogue chain runs
            for job in tail_fillers:
                job()

            # out-projection slice
            for jc in range(8):
                fp = (oap if jc % 2 == 0 else obp).tile(
                    [P, QB], F32, tag="oA" if jc % 2 == 0 else "oB", name="fp")
                nc.tensor.matmul(fp, wo_sb[:, jc * P : (jc + 1) * P], onT,
                                 start=True, stop=True)
                fs = fsp.tile([P, QB], BF16, tag="fs")
                if jc % 2 == 0:
                    nc.scalar.copy(out=fs, in_=fp)
                else:
                    nc.vector.tensor_copy(out=fs, in_=fp)
                nc.sync.dma_start(
                    out=y[jc * P : (jc + 1) * P, qsb * QB : (qsb + 1) * QB],
                    in_=fs)

        # pipeline: proj(0) upfront; then attn(i) with proj(i+1)'s q/k work
        # spread between chunks and its V^T work after the epilogue.
        for job in proj_fillers(0):
            job()
        for it_ in range(NQB):
            if it_ + 1 < NQB:
                pf = proj_fillers(it_ + 1)
                mid, tail = pf[:6], pf[6:]
            else:
                mid, tail = [], []
            emit_attn(it_, mid, tail)

    nc.compile()
    return nc


def _host_prep(x, token_positions, Wq, Wk, Wv, Wo):
    import ml_dtypes

    bf16 = ml_dtypes.bfloat16
    x = np.asarray(x, dtype=np.float32)
    pos = np.asarray(token_positions).astype(np.float32)
    Wq = np.asarray(Wq, dtype=np.float32) * np.float32(QSCALE)
    Wk = np.asarray(Wk, dtype=np.float32)
    Wv = np.asarray(Wv, dtype=np.float32)
    Wo = np.asarray(Wo, dtype=np.float32)

    xT = np.ascontiguousarray(x.reshape(S, D).T)  # [D, S]

    freqs = (1.0 / THETA ** (np.arange(0, HD, 2, dtype=np.float32) / HD)).astype(
        np.float32)
    ang = pos[:, None] * freqs[None, :]          # [S, 32]
    cosv = np.cos(ang).astype(np.float32).T      # [32, S]
    sinv = np.sin(ang).astype(np.float32).T
    C64 = np.repeat(cosv, 2, axis=0)             # [64, S]
    S64 = np.empty((HD, S), dtype=np.float32)
    S64[0::2] = -sinv
    S64[1::2] = sinv
    C = np.tile(C64, (2, 1))                     # [128, S]
    Sg = np.tile(S64, (2, 1))

    pswap = np.zeros((P, P), dtype=np.float32)
    idx = np.arange(P)
    pswap[idx ^ 1, idx] = 1.0
    tri = np.triu(np.ones((P, P), dtype=np.float32))
    cst = np.concatenate([pswap, tri], axis=1)   # [128, 256]

    def b(a):
        return np.ascontiguousarray(a).astype(bf16)

    in_maps = []
    for c in range(NCORE):
        r = slice(c * P, (c + 1) * P)
        in_maps.append({
            "xT": b(xT),
            "wq": b(Wq[r, :].T),
            "wk": b(Wk[r, :].T),
            "wv": b(Wv[r, :].T),
            "wo": b(Wo[:, r].T),
            "cs": b(C),
            "sn": b(Sg),
            "cst": b(cst),
        })
    return in_maps


LAST_EXEC_NS = None
LAST_TRACE = None


def kernel(x, token_positions, Wq, Wk, Wv, Wo):
    global LAST_EXEC_NS, LAST_TRACE
    from concourse.bass_utils import run_bass_kernel_spmd

    if "nc" not in _NC_CACHE:
        _NC_CACHE["nc"] = _build_nc()
    nc = _NC_CACHE["nc"]

    in_maps = _host_prep(x, token_positions, Wq, Wk, Wv, Wo)
    res = run_bass_kernel_spmd(nc, in_maps, core_ids=list(range(NCORE)))
    LAST_EXEC_NS = res.exec_time_ns
    LAST_TRACE = (
        res.instructions_and_trace[1]
        if res.instructions_and_trace is not None
        else None
    )

    acc = np.zeros((D, S), dtype=np.float32)
    for r in res.results:
        acc += r["y"].astype(np.float32)
    out = acc.T.astype(np.float32).reshape(1, S, D)
    return out


# revision 23
# speedup vs baseline: 1.3409x; 1.1260x over previous
"""Causal multi-head attention with RoPE on Trainium2, 8 NeuronCores.

Head-parallel sharding: 16 heads / 8 cores = 2 heads per core. Each core:
q/k/v projections for its 2 heads (128 of 1024 hidden dims), flash-style
causal attention in transposed-score layout (keys on partitions, softmax
denominator from an appended ones-column in V), row-parallel slice of the
output projection. Host sums the 8 partial (D,S) outputs.

v3 over v2:
- out-projection matmuls of block i are interleaved between the projection
  matmul groups of block i+1, so the PE never idles across a block boundary
  (kills the HAM re-throttle that cost ~100us at half clock).
- engine diet: tri-mask muls moved DVE->GpSimd; RoPE cos/sin muls run in
  bf16 (2x DVE mode); q/k raw copies moved DVE->ACT; softmax reciprocal
  taken directly from the PSUM l-row (drops 2 ACT copies/block); V^T
  PSUM->SBUF evacuation as one strided ACT copy per key chunk (both heads).
- y output in bf16 (halves the DMA write), host sums partials in f32.

Self-contained: hardcodes B=1, S=4096, D=1024, H=16, hd=64.
"""

import sys

if "/opt/trn_rl_repo" not in sys.path:
    sys.path.insert(0, "/opt/trn_rl_repo")

import numpy as np

S = 4096
D = 1024
H = 16
HD = 64
NCORE = 8
P = 128
QB = 512          # query block width
NQB = S // QB     # 8
KC = 128          # key chunk
THETA = 10000.0
PEND = 10         # AV lag (chunks) behind exp; also >= #fp filler slots

# exp split (block-local): DVE takes every 3rd chunk, but none of the last
# TAIL_ACT chunks of a block -- the DVE queue must be short at block end so
# the next block's RoPE adds and the epilogue reciprocal clear quickly.
DVE_EXP_MOD = 3
DVE_EXP_PHASE = 1
TAIL_ACT = 5

# custom exp constants (deg-4 minimax for e^t on [-0.4, 0.4], leading=1)
EB3 = 4.069521
EB2 = 12.099517
EB1 = 24.194042
EB0 = 24.195307
EA4 = 0.041330267
QSCALE = 0.125 / 64.0   # folded into Wq on host
SWAP_MASK = [i ^ 1 for i in range(32)]

_NC_CACHE = {}


def _register_custom_ops():
    from concourse.dve_spec import Spec, Src0, C0, C1, C2, lower
    from concourse.dve_uop import DveOpSpec
    from concourse import dve_ops as dvo

    def reg(name, spec):
        if name in dvo._SUB_OPCODE_FOR_NAME:
            return next(o for o in dvo.OPS if o.name == name)
        shas = {}
        for ver in ("v3",):
            uops = lower(spec, ver=ver)
            shas[ver] = DveOpSpec(name=name, opcode=0, uops=uops,
                                  rd1_en=False).sha(ver)
        op = dvo.DveOp(name, spec, subdim=False, uops_sha=shas)
        dvo.OPS.append(op)
        dvo.CUSTOM_DVE_SPECS[name] = spec
        dvo._SUB_OPCODE_FOR_NAME[name] = dvo._CUSTOM_DVE_ROW_BASE + len(dvo.OPS) - 1
        assert dvo._SUB_OPCODE_FOR_NAME[name] < 0x20
        return op

    f = np.float32
    poly = reg("EXP_POLY3M", Spec(
        body=(((Src0 + C0) * Src0 + C1) * Src0 + C2) * Src0,
        reference=lambda in0, in1, s0, s1, imm2: (
            (((in0.astype(np.float32) + f(s0)) * in0 + f(s1)) * in0 + f(imm2))
            * in0).astype(np.float32)))

    _m = (Src0 + C1) * C0
    _b = _m * _m
    for _ in range(5):
        _b = _b * _b

    def _refp(in0, in1, s0, s1, imm2):
        x = ((in0.astype(np.float32) + f(s1)) * f(s0)).astype(np.float32)
        for _ in range(6):
            x = (x * x).astype(np.float32)
        return x

    pw = reg("ADD_SCALE_POW64", Spec(body=_b, reference=_refp))
    return poly, pw


def _build_nc():
    import concourse.bacc as bacc
    import concourse.mybir as mybir
    from concourse.tile import TileContext
    from contextlib import ExitStack

    EXP_POLY3M, ADD_SCALE_POW64 = _register_custom_ops()

    F32 = mybir.dt.float32
    BF16 = mybir.dt.bfloat16
    EXP = mybir.ActivationFunctionType.Exp

    nc = bacc.Bacc("TRN2", target_bir_lowering=False)

    xT = nc.dram_tensor("xT", [D, S], BF16, kind="ExternalInput")
    wq = nc.dram_tensor("wq", [D, P], BF16, kind="ExternalInput")
    wk = nc.dram_tensor("wk", [D, P], BF16, kind="ExternalInput")
    wv = nc.dram_tensor("wv", [D, P], BF16, kind="ExternalInput")
    wo = nc.dram_tensor("wo", [P, D], BF16, kind="ExternalInput")
    cs = nc.dram_tensor("cs", [P, S], BF16, kind="ExternalInput")
    sn = nc.dram_tensor("sn", [P, S], BF16, kind="ExternalInput")
    cst = nc.dram_tensor("cst", [P, 2 * P], BF16, kind="ExternalInput")
    y = nc.dram_tensor("y", [D, S], BF16, kind="ExternalOutput")

    xTr = xT.rearrange("(o p) s -> p o s", p=P)   # [128, 8, 4096]
    wqr = wq.rearrange("(o p) m -> p o m", p=P)   # [128, 8, 128]
    wkr = wk.rearrange("(o p) m -> p o m", p=P)
    wvr = wv.rearrange("(o p) m -> p o m", p=P)

    with TileContext(nc) as tc, ExitStack() as ctx:
        con = ctx.enter_context(tc.tile_pool(name="con", bufs=1))
        xp = ctx.enter_context(tc.tile_pool(name="xp", bufs=10))
        tp = ctx.enter_context(tc.tile_pool(name="tp", bufs=3))
        ptp = ctx.enter_context(tc.tile_pool(name="ptp", bufs=PEND + 2))
        plp = ctx.enter_context(tc.tile_pool(name="plp", bufs=3))
        onp_ = ctx.enter_context(tc.tile_pool(name="onp", bufs=2))
        rlp = ctx.enter_context(tc.tile_pool(name="rlp", bufs=2))
        fsp = ctx.enter_context(tc.tile_pool(name="fsp", bufs=4))
        pp = ctx.enter_context(tc.tile_pool(name="pp", bufs=2, space="PSUM"))
        scp = ctx.enter_context(tc.tile_pool(name="scp", bufs=2, space="PSUM"))
        oap = ctx.enter_context(tc.tile_pool(name="oap", bufs=1, space="PSUM"))
        obp = ctx.enter_context(tc.tile_pool(name="obp", bufs=1, space="PSUM"))

        # ---- constants / weights ----
        wq_sb = con.tile([P, 8, P], BF16)
        nc.sync.dma_start(out=wq_sb, in_=wqr)
        wk_sb = con.tile([P, 8, P], BF16)
        nc.sync.dma_start(out=wk_sb, in_=wkr)
        wv_sb = con.tile([P, 8, P], BF16)
        nc.gpsimd.dma_start(out=wv_sb, in_=wvr)
        wo_sb = con.tile([P, D], BF16)
        nc.scalar.dma_start(out=wo_sb, in_=wo[:, :])
        cs_sb = con.tile([P, S], BF16)
        sn_sb = con.tile([P, S], BF16)
        nc.sync.dma_start(out=cs_sb[:, 0:QB], in_=cs[:, 0:QB])
        nc.sync.dma_start(out=sn_sb[:, 0:QB], in_=sn[:, 0:QB])
        nc.scalar.dma_start(out=cs_sb[:, QB:S], in_=cs[:, QB:S])
        nc.scalar.dma_start(out=sn_sb[:, QB:S], in_=sn[:, QB:S])
        cst_sb = con.tile([P, 2, P], BF16)
        nc.sync.dma_start(out=cst_sb, in_=cst.rearrange("p (t m) -> p t m", t=2))
        pswap_sb = cst_sb[:, 0, :]
        tri_sb = cst_sb[:, 1, :]

        # v in [key, hd] layout, both heads + ones column at index HD
        # layout [keys, chunk, head, HD+1]; AV lhsT = vna[:, ci, h, 0:HD+1]
        vna = con.tile([P, S // KC, 2, HD + 1], BF16, tag="vna", name="vna")
        nc.vector.memset(vna[:, :, :, HD : HD + 1], 1.0)

        qTr = con.tile([P, S], BF16, tag="qTr")
        kTr = con.tile([P, S], BF16, tag="kTr")

        # HAM warmup: dependency-free matmuls on scratch tiles keep the PE
        # busy (and its clock at 8/8) while the first DMAs land.
        wsc_w = con.tile([P, P], BF16, tag="wscw")
        wsc_x = con.tile([P, QB], BF16, tag="wscx")
        nc.vector.memset(wsc_w, 1.0)
        nc.vector.memset(wsc_x, 1.0)
        wups = oap.tile([P, QB], F32, tag="oA", name="warm")
        for _ in range(40):
            nc.tensor.matmul(wups, wsc_w, wsc_x, start=True, stop=True)

        def proj_fillers(st):
            """Projection work for block st as a list of PE-filler closures,
            to be interleaved between attention chunks of block st-1."""
            sl = slice(st * QB, (st + 1) * QB)
            xts = []
            for dk in range(8):
                xt = xp.tile([P, QB], BF16, tag="x")
                nc.sync.dma_start(out=xt, in_=xTr[:, dk, sl])
                xts.append(xt)

            state = {}

            def mk_acc(key, wsb, half):
                def job():
                    if half == 0:
                        state[key] = pp.tile([P, QB], F32, tag="ps",
                                             name=f"acc{key}")
                    acc = state[key]
                    for dk in range(4 * half, 4 * half + 4):
                        nc.tensor.matmul(acc, wsb[:, dk, :], xts[dk],
                                         start=(dk == 0), stop=(dk == 7))
                    if half == 1:
                        raw = tp.tile([P, QB], BF16, tag="raw",
                                      name=f"raw{key}")
                        nc.scalar.copy(out=raw, in_=acc)
                        state[f"raw{key}"] = raw
                return job

            def mk_rope(key, dstT):
                # pair-swap via DVE quadrant shuffle (i^1 is within-quadrant),
                # all-bf16 muls/adds run in the DVE 2x mode; no PSUM tiles.
                def job():
                    raw = state[f"raw{key}"]
                    t1 = tp.tile([P, QB], BF16, tag=f"t1{key}",
                                 name=f"t1{key}")
                    nc.vector.tensor_mul(out=t1, in0=raw, in1=cs_sb[:, sl])
                    shuf = tp.tile([P, QB], BF16, tag="shuf")
                    nc.vector.stream_shuffle(out=shuf, in_=raw, mask=SWAP_MASK)
                    t2 = tp.tile([P, QB], BF16, tag="t2")
                    nc.vector.tensor_mul(out=t2, in0=shuf, in1=sn_sb[:, sl])
                    nc.vector.tensor_add(out=dstT[:, sl], in0=t1, in1=t2)
                return job

            def mk_vac(sub):
                def job():
                    vac = pp.tile([P, QB], F32, tag="ps", name="vac")[:, 0:KC]
                    for dk in range(8):
                        nc.tensor.matmul(vac,
                                         xts[dk][:, sub * KC : (sub + 1) * KC],
                                         wv_sb[:, dk, :],
                                         start=(dk == 0), stop=(dk == 7))
                    ci = st * 4 + sub
                    nc.scalar.copy(out=vna[:, ci, :, 0:HD], in_=vac[:, 0:P])
                return job

            return ([mk_acc("q", wq_sb, 0), mk_acc("q", wq_sb, 1),
                     mk_rope("q", qTr),
                     mk_acc("k", wk_sb, 0), mk_acc("k", wk_sb, 1),
                     mk_rope("k", kTr)]
                    + [mk_vac(s) for s in range(4)])

        chunk_counter = [0]

        def emit_attn(qsb, fillers, tail_fillers, fp_prev):
            """Scores + exp + AV for query block qsb. `fillers` (next block's
            q/k proj + RoPE) are interleaved between chunks so the PE stream
            stays dense; `fp_prev` (previous block's out-projection) rides in
            chunks 2..9 (before the first AV flush, so the oA/oB PSUM ring
            order stays acyclic); `tail_fillers` (next block's V^T) are
            emitted after the epilogue. Returns this block's out-proj jobs."""
            nch = 4 * (qsb + 1)
            fillers = list(fillers)
            fpj = list(fp_prev)
            hold = {}
            pend = []

            def get_oAB():
                if "oA" not in hold:
                    hold["oA"] = oap.tile([P, QB], F32, tag="oA", name="oA")
                    hold["oB"] = obp.tile([P, QB], F32, tag="oB", name="oB")
                return hold["oA"], hold["oB"]

            def flush_av(last):
                oA, oB = get_oAB()
                ppt, poff, pc = pend.pop(0)
                for h, o in ((0, oA), (1, oB)):
                    nc.tensor.matmul(
                        o[0 : HD + 1, poff:QB],
                        vna[:, pc, h, :],
                        ppt[:, h, poff:QB],
                        start=(pc == 0), stop=last and (pc == nch - 1),
                    )

            for c in range(nch):
                # one filler every 2 chunks: early enough that qTr/kTr for
                # the next block are ready well before its first scores, late
                # enough that the RoPE DVE ops don't head-of-line-block the
                # DVE queue behind un-executed producers.
                if c % 2 == 0 and fillers:
                    fillers.pop(0)()
                if c >= 2 and fpj:
                    fpj.pop(0)()
                is_diag = (c // 4) == qsb
                off = (c % 4) * KC if is_diag else 0
                sp = scp.tile([P, 2, QB], F32, tag="sc")
                for h in (0, 1):
                    nc.tensor.matmul(
                        sp[:, h, off:QB],
                        kTr[h * HD : (h + 1) * HD, c * KC : (c + 1) * KC],
                        qTr[h * HD : (h + 1) * HD, qsb * QB + off : (qsb + 1) * QB],
                        start=True, stop=True,
                        tile_position=(h * HD, 0),
                    )
                pt = ptp.tile([P, 2, QB], BF16, tag="pt")
                gc = chunk_counter[0]
                chunk_counter[0] += 1
                if ((not is_diag) and c % DVE_EXP_MOD == DVE_EXP_PHASE
                        and c < nch - TAIL_ACT):
                    pl = plp.tile([P, 2, QB], F32, tag="pl")
                    nc.vector._custom_dve(
                        EXP_POLY3M, out=pl[:, :, off:QB], in0=sp[:, :, off:QB],
                        s0=EB3, s1=EB2, imm2=EB1)
                    nc.vector._custom_dve(
                        ADD_SCALE_POW64, out=pt[:, :, off:QB], in0=pl[:, :, off:QB],
                        s0=EA4, s1=EB0)
                else:
                    nc.scalar.activation(
                        out=pt[:, :, off:QB], in_=sp[:, :, off:QB], func=EXP,
                        scale=64.0)
                if is_diag:
                    for h in (0, 1):
                        nc.gpsimd.tensor_mul(
                            out=pt[:, h, off : off + KC],
                            in0=pt[:, h, off : off + KC],
                            in1=tri_sb,
                        )
                if len(pend) == PEND:
                    flush_av(False)
                pend.append((pt, off, c))
            while fillers:
                fillers.pop(0)()
            while fpj:
                fpj.pop(0)()
            while pend:
                flush_av(True)
            oA, oB = get_oAB()

            # epilogue: l-rows to SBUF on ACT (cross-partition copy), DVE
            # reciprocal on the [1,QB] rows, gpsimd broadcast of the recip.
            rlA = rlp.tile([1, QB], F32, tag="rlA")
            nc.scalar.copy(out=rlA, in_=oA[HD : HD + 1, :])
            rlB = rlp.tile([1, QB], F32, tag="rlB")
            nc.scalar.copy(out=rlB, in_=oB[HD : HD + 1, :])
            rA = rlp.tile([1, QB], F32, tag="rA")
            nc.vector.reciprocal_approx_fast(out=rA, in_=rlA)
            rB = rlp.tile([1, QB], F32, tag="rB")
            nc.vector.reciprocal_approx_fast(out=rB, in_=rlB)
            rlbA = onp_.tile([HD, QB], F32, tag="rlb")
            nc.gpsimd.partition_broadcast(rlbA[0:HD, :], rA[0:1, :])
            rlbB = onp_.tile([HD, QB], F32, tag="rlb")
            nc.gpsimd.partition_broadcast(rlbB[0:HD, :], rB[0:1, :])

            onT = onp_.tile([P, QB], BF16, tag="onT")
            nc.vector.tensor_mul(out=onT[0:HD, :], in0=oA[0:HD, :], in1=rlbA[0:HD, :])
            oBn = onp_.tile([HD, QB], BF16, tag="oBn")
            nc.vector.tensor_mul(out=oBn[0:HD, :], in0=oB[0:HD, :], in1=rlbB[0:HD, :])
            nc.gpsimd.dma_start(out=onT[HD : 2 * HD, :], in_=oBn[0:HD, :])

            # next block's V^T fills the PE while the epilogue chain runs
            for job in tail_fillers:
                job()

            # out-projection jobs: ride in the next block's chunk stream
            def mk_fp(jc):
                def job():
                    fp = (oap if jc % 2 == 0 else obp).tile(
                        [P, QB], F32, tag="oA" if jc % 2 == 0 else "oB",
                        name="fp")
                    nc.tensor.matmul(fp, wo_sb[:, jc * P : (jc + 1) * P], onT,
                                     start=True, stop=True)
                    fs = fsp.tile([P, QB], BF16, tag="fs")
                    if jc % 2 == 0:
                        nc.scalar.copy(out=fs, in_=fp)
                    else:
                        nc.vector.tensor_copy(out=fs, in_=fp)
                    nc.sync.dma_start(
                        out=y[jc * P : (jc + 1) * P,
                              qsb * QB : (qsb + 1) * QB],
                        in_=fs)
                return job

            return [mk_fp(jc) for jc in range(8)]

        # pipeline: proj(0) upfront; then attn(i) with proj(i+1)'s q/k work
        # spread between chunks and its V^T work after the epilogue.
        for job in proj_fillers(0):
            job()
        fp_prev = []
        for it_ in range(NQB):
            if it_ + 1 < NQB:
                pf = proj_fillers(it_ + 1)
                mid, tail = pf[:6], pf[6:]
            else:
                mid, tail = [], []
            fp_prev = emit_attn(it_, mid, tail, fp_prev)
        for job in fp_prev:
            job()

    nc.compile()
    return nc


def _host_prep(x, token_positions, Wq, Wk, Wv, Wo):
    import ml_dtypes

    bf16 = ml_dtypes.bfloat16
    x = np.asarray(x, dtype=np.float32)
    pos = np.asarray(token_positions).astype(np.float32)
    Wq = np.asarray(Wq, dtype=np.float32) * np.float32(QSCALE)
    Wk = np.asarray(Wk, dtype=np.float32)
    Wv = np.asarray(Wv, dtype=np.float32)
    Wo = np.asarray(Wo, dtype=np.float32)

    xT = np.ascontiguousarray(x.reshape(S, D).T)  # [D, S]

    freqs = (1.0 / THETA ** (np.arange(0, HD, 2, dtype=np.float32) / HD)).astype(
        np.float32)
    ang = pos[:, None] * freqs[None, :]          # [S, 32]
    cosv = np.cos(ang).astype(np.float32).T      # [32, S]
    sinv = np.sin(ang).astype(np.float32).T
    C64 = np.repeat(cosv, 2, axis=0)             # [64, S]
    S64 = np.empty((HD, S), dtype=np.float32)
    S64[0::2] = -sinv
    S64[1::2] = sinv
    C = np.tile(C64, (2, 1))                     # [128, S]
    Sg = np.tile(S64, (2, 1))

    pswap = np.zeros((P, P), dtype=np.float32)
    idx = np.arange(P)
    pswap[idx ^ 1, idx] = 1.0
    tri = np.triu(np.ones((P, P), dtype=np.float32))
    cst = np.concatenate([pswap, tri], axis=1)   # [128, 256]

    def b(a):
        return np.ascontiguousarray(a).astype(bf16)

    in_maps = []
    for c in range(NCORE):
        r = slice(c * P, (c + 1) * P)
        in_maps.append({
            "xT": b(xT),
            "wq": b(Wq[r, :].T),
            "wk": b(Wk[r, :].T),
            "wv": b(Wv[r, :].T),
            "wo": b(Wo[:, r].T),
            "cs": b(C),
            "sn": b(Sg),
            "cst": b(cst),
        })
    return in_maps


LAST_EXEC_NS = None
LAST_TRACE = None


def kernel(x, token_positions, Wq, Wk, Wv, Wo):
    global LAST_EXEC_NS, LAST_TRACE
    from concourse.bass_utils import run_bass_kernel_spmd

    if "nc" not in _NC_CACHE:
        _NC_CACHE["nc"] = _build_nc()
    nc = _NC_CACHE["nc"]

    in_maps = _host_prep(x, token_positions, Wq, Wk, Wv, Wo)
    res = run_bass_kernel_spmd(nc, in_maps, core_ids=list(range(NCORE)))
    LAST_EXEC_NS = res.exec_time_ns
    LAST_TRACE = (
        res.instructions_and_trace[1]
        if res.instructions_and_trace is not None
        else None
    )

    acc = np.zeros((D, S), dtype=np.float32)
    for r in res.results:
        acc += r["y"].astype(np.float32)
    out = acc.T.astype(np.float32).reshape(1, S, D)
    return out
